# revision 21
# baseline (speedup 1.0000x reference)
"""GQA attention layer (B=4, S=2048, D=2048, 16 heads / 4 KV heads, RoPE,
causal) on 8 trn2 NeuronCores.

Sharding: TP=4 over KV-head groups x DP=2 over batch. Each core handles 2
batches and one KV group (4 q heads + 1 kv head), computes a partial
(head-group) contribution to out = attn @ wo; host sums the 4 partials per
batch group.

Device layout choices:
  - host pre-casts to bf16, packs x/wq/wk/wv/wo so every SBUF load is one
    contiguous 4-16KB run per partition (128 DMA descriptors per tensor
    instead of ~2048), and permutes wq/wk columns per head to "evens then
    odds" so RoPE becomes rotate-half.
  - q,k are produced transposed ([dh, tok]) straight from the projection
    matmuls; v is produced natural ([tok, dh]).
  - RoPE in transposed layout: rot = raw * C2 + swap_halves(raw) * S2 with
    C2 = [cos;cos], S2 = [-sin;+sin]; the half swap is folded into two
    half-partition DVE multiplies (no SBUF->SBUF copy).
  - attention: scoresT = kT_tile.T @ qT (k on partitions), exp on ACT (no
    max subtraction -- scores are O(5) here). Causal masking: the only
    invalid entries live in the 128x128 diagonal block of each masked
    k-tile; GPSIMD affine_select zeroes them in the exp tile (iota
    q_local - k >= 0), so no mask matmuls on PE. All ops on a diagonal
    tile skip its fully-masked first 128*r columns.
    PV matmuls accumulate in PSUM over k-tiles. Row sums: exp tiles are
    pair-added on DVE in groups of 4 (bf16) and a single ones @ quad_sum
    matmul per quad accumulates l in PSUM -- 4x fewer PE row-sum matmuls.
    1/l via ACT ln -> exp(-x) (both in one activation-table set).
"""

import math
from contextlib import ExitStack

import ml_dtypes
import numpy as np

import concourse.bass as bass
import concourse.mybir as mybir
import concourse.tile as tile
from concourse import bacc
from concourse.bass_utils import run_bass_kernel_spmd

BF16 = mybir.dt.bfloat16
F32 = mybir.dt.float32

# Full-problem constants (hardcoded per harness contract)
B, S, D = 4, 2048, 2048
NH, NKV, DH = 16, 4, 128
TP, DP = 4, 2
BL = B // DP          # batches per core
T = BL * S            # tokens per core
HL = NH // TP         # q heads per core
QC = HL * DH          # q cols per core
NT128 = S // 128      # 128-token tiles per batch (16)
NSL = S // 512        # 512-token slices per batch (4)
KD = D // 128         # contraction tiles for the projections (16)
NSI = T // 512        # 512-token x slices per core (8)


def _patch_act_tables():
    """Make natural_log_exp_and_others the only set claiming Exp/Ln so the
    act-table-load pass emits one load instead of thrashing between the
    exp-only and ln-only sets on every softmax-reciprocal."""
    if getattr(bacc, "_act_tables_patched", False):
        return
    orig = bacc.get_activation_tables

    def patched(arch):
        tabs = orig(arch)
        both = tabs.get("natural_log_exp_and_others")
        if both is None:
            return tabs
        exp = mybir.ActivationFunctionType.Exp
        ln = mybir.ActivationFunctionType.Ln
        for name, s in tabs.items():
            if name != "natural_log_exp_and_others":
                s.discard(exp)
                s.discard(ln)
        return tabs

    bacc.get_activation_tables = patched
    bacc._act_tables_patched = True


def build_nc(sc_bufs=3, oT_bufs=2, wo_bufs=2, exp_bufs=6, xt_bufs=2,
             q_bufs=2, k_bufs=2, v_bufs=2, wo_copy="vec",
             psb_bufs=2, asb_bufs=3, mask_mode="gps"):
    _patch_act_tables()
    nc = bacc.Bacc("TRN2", target_bir_lowering=False, debug=False)

    xt = nc.dram_tensor("xt", [128, NSI, KD, 512], BF16, kind="ExternalInput").ap()
    wq = nc.dram_tensor("wq", [128, HL, KD, DH], BF16, kind="ExternalInput").ap()
    wk = nc.dram_tensor("wk", [128, KD, DH], BF16, kind="ExternalInput").ap()
    wv = nc.dram_tensor("wv", [128, KD, DH], BF16, kind="ExternalInput").ap()
    wo = nc.dram_tensor("wo", [128, HL, D], BF16, kind="ExternalInput").ap()
    cos2 = nc.dram_tensor("cos2", [DH, S], BF16, kind="ExternalInput").ap()
    sin2 = nc.dram_tensor("sin2", [DH, S], BF16, kind="ExternalInput").ap()
    out = nc.dram_tensor("out", [T, D], F32, kind="ExternalOutput").ap()

    scale = 1.0 / math.sqrt(DH)

    with tile.TileContext(nc) as tc, ExitStack() as ctx:
        persist = ctx.enter_context(tc.tile_pool(name="persist", bufs=1))

        # --- resident weights / tables (contiguous per-partition DMAs).
        # Issue order = DMA queue order: wk/wv first (first matmuls need
        # them), the 2MB wq and wo after the first x slice, wo last (not
        # needed until phase C) ---
        wk_sb = persist.tile([128, KD, DH], BF16, tag="wk")
        nc.sync.dma_start(wk_sb[:], wk)
        wv_sb = persist.tile([128, KD, DH], BF16, tag="wv")
        nc.sync.dma_start(wv_sb[:], wv)
        cos_sb = persist.tile([128, S], BF16, tag="cos")
        sin_sb = persist.tile([128, S], BF16, tag="sin")
        wq_sb = persist.tile([128, HL, KD, DH], BF16, tag="wq")
        wo_sb = persist.tile([128, HL, D], BF16, tag="wo")
        ones_sb = persist.tile([128, 128], BF16, tag="ones")
        nc.vector.memset(ones_sb[:], 1.0)

        # --- resident activations ---
        qT_sb = persist.tile([128, HL, BL, S], BF16, tag="qT")
        kT_sb = persist.tile([128, BL, S], BF16, tag="kT")
        v_sb = persist.tile([128, BL, NT128, DH], BF16, tag="v")

        # SBUF pools all stay open for the whole kernel so phase C tiles
        # don't alias phase B tiles (aliasing adds false WAR deps that
        # serialize the phases). Only PSUM pools (8 banks) are scoped.
        psb = ctx.enter_context(tc.tile_pool(name="proj_sb", bufs=psb_bufs))
        asb = ctx.enter_context(tc.tile_pool(name="att_sb", bufs=asb_bufs))
        asb2 = ctx.enter_context(tc.tile_pool(name="att_sb2", bufs=2))

        # ---------------- phase B: projections + RoPE ----------------
        with tc.tile_pool(name="proj_ps", bufs=2, space="PSUM") as pps:

            def rope(dst, raw_ps, pos_sl):
                """dst[128,512] <- RoPE(raw_ps[128,512] psum), via bf16 sbuf."""
                raw = psb.tile([128, 512], BF16, tag="rraw")
                nc.scalar.copy(raw[:], raw_ps[:])
                swp = psb.tile([128, 512], BF16, tag="rswp")
                nc.sync.dma_start(swp[0:64, :], raw[64:128, :])
                nc.sync.dma_start(swp[64:128, :], raw[0:64, :])
                t1 = psb.tile([128, 512], BF16, tag="rt1")
                nc.vector.tensor_mul(t1[:], raw[:], cos_sb[:, pos_sl])
                t2 = psb.tile([128, 512], BF16, tag="rt2")
                nc.vector.tensor_mul(t2[:], swp[:], sin_sb[:, pos_sl])
                nc.vector.tensor_add(dst, t1[:], t2[:])

            for si in range(NSI):
                b, sl = divmod(si, NSL)
                pos_sl = bass.ts(sl, 512)
                xt_sl = psb.tile([128, KD, 512], BF16, tag="xt", bufs=xt_bufs)
                if si == 0:
                    # chunk the first x slice so the k-projection starts
                    # after 1/4 of it has landed; wq arrives per-head so
                    # the first q matmuls don't wait for the full 2MB
                    for ch in range(4):
                        nc.sync.dma_start(
                            xt_sl[:, bass.ts(ch, 4)], xt[:, si, bass.ts(ch, 4)]
                        )
                    nc.sync.dma_start(cos_sb[:], cos2)
                    nc.sync.dma_start(sin_sb[:], sin2)
                    for h in range(HL):
                        nc.sync.dma_start(wq_sb[:, h], wq[:, h])
                else:
                    nc.sync.dma_start(xt_sl[:], xt[:, si])
                    if si == 1:
                        nc.sync.dma_start(wo_sb[:], wo)
                k_ps = pps.tile([128, 512], F32, tag="k", bufs=k_bufs)
                for o in range(KD):
                    nc.tensor.matmul(
                        k_ps[:], wk_sb[:, o, :], xt_sl[:, o, :],
                        start=(o == 0), stop=(o == KD - 1),
                    )
                rope(kT_sb[:, b, pos_sl], k_ps, pos_sl)
                for jt in range(4):
                    v_ps = pps.tile([128, DH], F32, tag="v", bufs=v_bufs)
                    for o in range(KD):
                        nc.tensor.matmul(
                            v_ps[:], xt_sl[:, o, bass.ts(jt, 128)], wv_sb[:, o, :],
                            start=(o == 0), stop=(o == KD - 1),
                        )
                    nc.scalar.copy(v_sb[:, b, 4 * sl + jt, :], v_ps[:])
                for h in range(HL):
                    q_ps = pps.tile([128, 512], F32, tag="q", bufs=q_bufs)
                    for o in range(KD):
                        nc.tensor.matmul(
                            q_ps[:], wq_sb[:, h, o, :], xt_sl[:, o, :],
                            start=(o == 0), stop=(o == KD - 1),
                        )
                    rope(qT_sb[:, h, b, pos_sl], q_ps, pos_sl)

        # ---------------- phase C: attention + wo ----------------
        with tc.tile_pool(name="att_ps", bufs=2, space="PSUM") as aps:
            for b in range(BL):
                for qs in range(NSL):
                    q_sl = bass.ts(qs, 512)
                    nk = 4 * qs + 4
                    aoT = asb2.tile([128, HL, 512], BF16, tag="aoT")
                    for h in range(HL):
                        oT_ps = aps.tile([128, 512], F32, tag="oT", bufs=oT_bufs)
                        # bufs=1: keeps sc4+oT2+l1+wo1 = 8 PSUM banks
                        l_ps = aps.tile([128, 512], F32, tag="l", bufs=1)
                        # l groups: pairs of quad-sums (octets) plus a
                        # final unpaired quad when qs is even
                        ngroups = (qs + 2) // 2
                        lgi = 0
                        equad = []
                        quads = []
                        for j in range(nk):
                            r = j - 4 * qs
                            masked = r >= 0
                            # columns q_local < 128 r are fully masked for
                            # the r-th diagonal tile: skip them everywhere.
                            qlo = 128 * r if masked else 0
                            qg = bass.ds(qs * 512 + qlo, 512 - qlo)
                            s_ps = aps.tile([128, 512], F32, tag="sc", bufs=sc_bufs)
                            nc.tensor.matmul(
                                s_ps[:, qlo:], kT_sb[:, b, bass.ts(j, 128)],
                                qT_sb[:, h, b, qg], start=True, stop=True,
                            )
                            e_sb = asb.tile([128, 512], BF16, tag="exp", bufs=exp_bufs)
                            nc.scalar.activation(
                                e_sb[:, qlo:], s_ps[:, qlo:],
                                mybir.ActivationFunctionType.Exp, scale=scale,
                            )
                            if masked:
                                # zero the causally-invalid triangle of the
                                # 128x128 diagonal block: keep where
                                # q_local_in_block - k >= 0.
                                nc.gpsimd.affine_select(
                                    e_sb[:, qlo:qlo + 128],
                                    e_sb[:, qlo:qlo + 128],
                                    pattern=[[1, 128]],
                                    compare_op=mybir.AluOpType.is_ge,
                                    fill=0.0,
                                    base=0,
                                    channel_multiplier=-1,
                                )
                            nc.tensor.matmul(
                                oT_ps[:, qlo:], v_sb[:, b, j, :], e_sb[:, qlo:],
                                start=(j == 0), stop=(j == nk - 1),
                                skip_group_check=True,
                            )
                            equad.append((e_sb, qlo))
                            if len(equad) == 4:
                                g = j // 4
                                tq = asb.tile([128, 512], BF16, tag="tq", bufs=6)
                                if equad[-1][1] > 0:
                                    # diagonal quad: staircase valid ranges
                                    nc.vector.tensor_copy(tq[:], equad[0][0][:])
                                    for (eq, lo) in equad[1:]:
                                        nc.vector.tensor_add(
                                            tq[:, lo:], tq[:, lo:], eq[:, lo:]
                                        )
                                else:
                                    t01 = asb.tile([128, 512], BF16, tag="tp", bufs=4)
                                    nc.vector.tensor_add(
                                        t01[:], equad[0][0][:], equad[1][0][:]
                                    )
                                    t23 = asb.tile([128, 512], BF16, tag="tp", bufs=4)
                                    nc.vector.tensor_add(
                                        t23[:], equad[2][0][:], equad[3][0][:]
                                    )
                                    nc.vector.tensor_add(tq[:], t01[:], t23[:])
                                quads.append(tq)
                                equad = []
                                # fold pairs of quad-sums on DVE so l needs
                                # one ones-matmul per octet; issue each
                                # group's matmul as soon as it is ready
                                tg = None
                                if len(quads) == 2:
                                    t8 = asb.tile([128, 512], BF16, tag="tq", bufs=6)
                                    nc.vector.tensor_add(
                                        t8[:], quads[0][:], quads[1][:]
                                    )
                                    tg = t8
                                    quads = []
                                elif j == nk - 1:
                                    tg = quads.pop()
                                if tg is not None:
                                    nc.tensor.matmul(
                                        l_ps[:], ones_sb[:], tg[:],
                                        start=(lgi == 0), stop=(lgi == ngroups - 1),
                                        skip_group_check=True,
                                    )
                                    lgi += 1
                        rc_sb = asb.tile([128, 512], F32, tag="rc")
                        nc.vector.reciprocal_approx_fast(rc_sb[:], l_ps[:])
                        nc.vector.tensor_mul(aoT[:, h, :], oT_ps[:], rc_sb[:])
                    # wo for these 512 tokens
                    for nt in range(4):
                        for od in range(4):
                            w_ps = aps.tile([128, 512], F32, tag="wo", bufs=wo_bufs)
                            for c in range(HL):
                                nc.tensor.matmul(
                                    w_ps[:], aoT[:, c, bass.ts(nt, 128)],
                                    wo_sb[:, c, bass.ts(od, 512)],
                                    start=(c == 0), stop=(c == HL - 1),
                                )
                            o_sb = asb.tile([128, 512], F32, tag="out")
                            if wo_copy == "act" or (
                                wo_copy == "mix" and (nt + od) % 2 == 0
                            ):
                                nc.scalar.copy(o_sb[:], w_ps[:])
                            elif wo_copy == "gps":
                                nc.gpsimd.tensor_copy(o_sb[:], w_ps[:])
                            else:
                                nc.vector.tensor_copy(o_sb[:], w_ps[:])
                            nc.sync.dma_start(
                                out[
                                    bass.ds(b * S + qs * 512 + nt * 128, 128),
                                    bass.ts(od, 512),
                                ],
                                o_sb[:],
                            )
    nc.finalize()
    return nc


_NC_CACHE = {}


def _get_nc():
    if "nc" not in _NC_CACHE:
        _NC_CACHE["nc"] = build_nc()
    return _NC_CACHE["nc"]


def kernel(x, freqs_cos, freqs_sin, wq, wk, wv, wo):
    x = np.asarray(x)
    freqs_cos = np.asarray(freqs_cos)
    freqs_sin = np.asarray(freqs_sin)
    wq = np.asarray(wq)
    wk = np.asarray(wk)
    wv = np.asarray(wv)
    wo = np.asarray(wo)
    bf = ml_dtypes.bfloat16
    perm = np.concatenate([np.arange(0, DH, 2), np.arange(1, DH, 2)])

    wq_p = wq.reshape(D, NH, DH)[:, :, perm].reshape(D, NH * DH).astype(bf)
    wk_p = wk.reshape(D, NKV, DH)[:, :, perm].reshape(D, NKV * DH).astype(bf)
    wv_b = wv.astype(bf)
    wo_b = wo.astype(bf)

    cosT = freqs_cos.T  # [64, S]
    sinT = freqs_sin.T
    c2 = np.ascontiguousarray(np.concatenate([cosT, cosT], axis=0)).astype(bf)
    s2 = np.ascontiguousarray(np.concatenate([-sinT, sinT], axis=0)).astype(bf)

    # per-dp x, packed [p, si, o, t] so each si-slice is one contiguous
    # 16KB run per partition
    xts = []
    for dp in range(DP):
        xs = x[dp * BL: (dp + 1) * BL].reshape(T, D).astype(bf)
        xts.append(
            np.ascontiguousarray(
                xs.reshape(NSI, 512, KD, 128).transpose(3, 0, 2, 1)
            )
        )

    in_maps = []
    for core in range(8):
        dp, tp = divmod(core, TP)
        wq_s = wq_p[:, tp * QC: (tp + 1) * QC]
        wk_s = wk_p[:, tp * DH: (tp + 1) * DH]
        wv_s = wv_b[:, tp * DH: (tp + 1) * DH]
        wo_s = wo_b[tp * QC: (tp + 1) * QC, :]
        in_maps.append(
            {
                "xt": xts[dp],
                "wq": np.ascontiguousarray(
                    wq_s.reshape(KD, 128, HL, DH).transpose(1, 2, 0, 3)
                ),
                "wk": np.ascontiguousarray(
                    wk_s.reshape(KD, 128, DH).transpose(1, 0, 2)
                ),
                "wv": np.ascontiguousarray(
                    wv_s.reshape(KD, 128, DH).transpose(1, 0, 2)
                ),
                "wo": np.ascontiguousarray(
                    wo_s.reshape(HL, 128, D).transpose(1, 0, 2)
                ),
                "cos2": c2,
                "sin2": s2,
            }
        )

    nc = _get_nc()
    res = run_bass_kernel_spmd(nc, in_maps, core_ids=list(range(8)))
    _NC_CACHE["last_results"] = res

    full = np.zeros((B, S, D), dtype=np.float32)
    for core in range(8):
        dp = core // TP
        full[dp * BL: (dp + 1) * BL] += (
            res.results[core]["out"].astype(np.float32).reshape(BL, S, D)
        )
    return full
